# revision 10
# baseline (speedup 1.0000x reference)
"""GATv2 encoder (2-layer, 4-head, N=20000, E=160000) on 8 Trainium2 cores.

Strategy: 1D node partition (dst-sharded edges). Nodes are renumbered into
160 balanced blocks of 128 (20 blocks/core); each block's incoming edges are
grouped and padded to a multiple of 128. Per edge tile: gather h[src] rows,
compute s = xl[src]+xr[dst]+ee via PSUM-accumulated f32r matmuls, logits via
Prelu/att-dot, and aggregate exp-weighted sums per destination with 0/1
selection-matrix matmuls. Softmax normalization (division by the segment sum)
is applied at the node level after aggregation — mathematically identical to
the reference's segment softmax; the segment-max subtraction is skipped since
all logits are O(0.1) (exp cannot overflow, and +1e-16 is below fp32 ulp of
den>=1). Cross-core traffic: AllGather of the 10MB h table per layer plus two
tiny AllReduces (edge-feature mean, attention normalizer).
"""
import sys
sys.path.insert(0, "/opt/trn_rl_repo")
import numpy as np

import concourse.bass as bass
import concourse.mybir as mybir
import concourse.tile as tile
from concourse.masks import make_identity
from concourse.vector_clock import ScopedClock

# ---------------------------------------------------------------- constants
N = 20000
E = 160000
DIN = 384
HID = 128
HEADS = 4
HD = HEADS * HID  # 512
L = 2
NEG = 0.2
EPS_LN = 1e-5
NC = 8           # cores
P = 128
NB = 160         # node blocks total
NBC = NB // NC   # blocks per core = 20
NPC = NBC * P    # node slots per core = 2560
NTOT = NB * P    # 20480
KCH = DIN // P   # 3 k-chunks

F32 = mybir.dt.float32
F32R = mybir.dt.float32r
BF16 = mybir.dt.bfloat16
I32 = mybir.dt.int32
AF = mybir.ActivationFunctionType
OP = mybir.AluOpType
AX = mybir.AxisListType

# ------------------------------------------------------- walrus workarounds
_uid = [0]


def _drain_and_barrier_semonly(self, tick_clock, wait_clock):
    nc = self.nc
    probe = nc.sync.nop(nofuse=True)
    wait_clock.add_sem_waits(probe.ins, ScopedClock({None: tick_clock.global_clock}))
    nc.all_engine_barrier(sem_only=True)
    assert self.sems is not None
    popped = nc._tile_sem_poison_stack.pop()
    assert popped is self._sem_poison
    nc.clear_and_free_semaphores(list(self.sems.allocated().values()))
    nc.all_engine_barrier(sem_only=True)


tile.TileContext._drain_and_barrier = _drain_and_barrier_semonly


def _split_waits(nc):
    """This walrus supports at most one inline sync wait per instruction;
    move extra waits onto same-engine nops inserted before the instruction."""
    for bb in nc.main_func.blocks:
        new_insts = []
        for ins in bb.instructions:
            si = ins.sync_info
            if si is not None and len(si.on_wait) > 1:
                waits = list(si.on_wait)
                for w in waits[:-1]:
                    _uid[0] += 1
                    new_insts.append(
                        mybir.InstNoOp(
                            name=f"I-waitsplit-{_uid[0]}",
                            sync_info=mybir.SyncInfo(on_wait=[w], on_update=[]),
                            bass_nofuse=True,
                            engine=ins.engine,
                        )
                    )
                ins.sync_info = mybir.SyncInfo(
                    on_wait=waits[-1:], on_update=list(si.on_update)
                )
            new_insts.append(ins)
        bb.instructions[:] = new_insts


# ------------------------------------------------------------ device program
def build_program(ET):
    """ET = edge tiles per node block (uniform across blocks/cores)."""
    EPC = NBC * ET * P  # edge slots per core
    nc = bass.Bass()

    # inputs (per core)
    ea3_d = nc.dram_tensor("ea3", [P, KCH * EPC], F32R, kind="ExternalInput")
    src_d = nc.dram_tensor("srcs", [EPC], I32, kind="ExternalInput")
    dstl_d = nc.dram_tensor("dstl", [EPC], I32, kind="ExternalInput")
    xT_d = nc.dram_tensor("xT", [DIN, NPC], F32R, kind="ExternalInput")
    w_in_d = nc.dram_tensor("w_in", [DIN, HID], F32R, kind="ExternalInput")
    w_q_d = nc.dram_tensor("w_q", [DIN, HID], F32R, kind="ExternalInput")
    q_d = nc.dram_tensor("qv", [DIN, 1], F32R, kind="ExternalInput")
    bq_d = nc.dram_tensor("b_inq", [1, HID], F32, kind="ExternalInput")  # b_in+b_q... passed separately below
    w_l_d = nc.dram_tensor("w_l", [L, HID, HD], F32R, kind="ExternalInput")
    w_r_d = nc.dram_tensor("w_r", [L, HID, HD], F32R, kind="ExternalInput")
    w_e_d = nc.dram_tensor("w_e", [L, DIN, HD], F32R, kind="ExternalInput")
    att_d = nc.dram_tensor("attv", [L, HD], F32, kind="ExternalInput")
    blr_d = nc.dram_tensor("blr", [L, HD], F32, kind="ExternalInput")  # b_l+b_r rows
    gb_d = nc.dram_tensor("gb", [L, HID], F32, kind="ExternalInput")
    lng_d = nc.dram_tensor("lng", [L, HID], F32, kind="ExternalInput")
    lnb_d = nc.dram_tensor("lnb", [L, HID], F32, kind="ExternalInput")

    h_out_d = nc.dram_tensor("h_out", [NPC, HID], F32, kind="ExternalOutput")
    na_out_d = nc.dram_tensor("na_out", [NPC], F32, kind="ExternalOutput")

    groups = [list(range(NC))]

    with tile.TileContext(nc) as tc:
        with (
            tc.tile_pool(name="const", bufs=1) as cn,
            tc.tile_pool(name="pers", bufs=1) as pr,
            tc.tile_pool(name="work", bufs=3) as wk,
            tc.tile_pool(name="wide", bufs=3) as wd,
            tc.tile_pool(name="ps_s", bufs=2, space="PSUM") as ps_s,
            tc.tile_pool(name="ps_xl", bufs=2, space="PSUM") as ps_xl,
            tc.tile_pool(name="ps_num", bufs=1, space="PSUM") as ps_num,
            tc.tile_pool(name="ps_den", bufs=1, space="PSUM") as ps_den,
            tc.tile_pool(name="ps_tr", bufs=2, space="PSUM") as ps_tr,
            tc.tile_pool(name="dram", bufs=1, space="DRAM") as dram,
        ):
            # ---------------- constants / weights
            ident_f = cn.tile([P, P], F32)
            make_identity(nc, ident_f[:])
            ident = cn.tile([P, P], F32R)
            nc.vector.tensor_copy(out=ident[:], in_=ident_f[:])
            iota_row = cn.tile([P, P], I32)
            nc.gpsimd.iota(iota_row[:], pattern=[[1, P]], channel_multiplier=0)
            iota_col = cn.tile([P, P], I32)
            nc.gpsimd.iota(iota_col[:], pattern=[[0, P]], channel_multiplier=1)
            ones_row = cn.tile([1, P], F32)
            nc.gpsimd.memset(ones_row[:], 1.0)
            ones_row_r = cn.tile([1, P], F32R)
            nc.vector.tensor_copy(out=ones_row_r[:], in_=ones_row[:])
            oc_f = cn.tile([P, 1], F32)
            nc.gpsimd.memset(oc_f[:], 1.0)
            ones_col_r = cn.tile([P, 1], F32R)
            nc.vector.tensor_copy(out=ones_col_r[:], in_=oc_f[:])

            w_in = [cn.tile([P, HID], F32R, name=f"w_in{k}") for k in range(KCH)]
            w_q = [cn.tile([P, HID], F32R, name=f"w_q{k}") for k in range(KCH)]
            qv = [cn.tile([P, 1], F32R, name=f"qv{k}") for k in range(KCH)]
            for k in range(KCH):
                nc.sync.dma_start(out=w_in[k][:], in_=w_in_d[k * P:(k + 1) * P, :])
                nc.sync.dma_start(out=w_q[k][:], in_=w_q_d[k * P:(k + 1) * P, :])
                nc.sync.dma_start(out=qv[k][:], in_=q_d[k * P:(k + 1) * P, :])
            w_l = [cn.tile([P, HD], F32R, name=f"w_l{i}") for i in range(L)]
            w_r = [cn.tile([P, HD], F32R, name=f"w_r{i}") for i in range(L)]
            w_e = [[cn.tile([P, HD], F32R, name=f"w_e{i}_{k}") for k in range(KCH)] for i in range(L)]
            for i in range(L):
                nc.sync.dma_start(out=w_l[i][:], in_=w_l_d[i, :, :])
                nc.sync.dma_start(out=w_r[i][:], in_=w_r_d[i, :, :])
                for k in range(KCH):
                    nc.sync.dma_start(out=w_e[i][k][:], in_=w_e_d[i, k * P:(k + 1) * P, :])

            # persistent state
            h_node = pr.tile([P, NPC], F32)        # node-major h (block cols)
            hT = pr.tile([P, NPC], F32R)           # hid-major h
            num_all = pr.tile([P, NBC * HD], F32)  # per-block num
            den_all = pr.tile([P, NBC * 5], F32)
            attn_acc = pr.tile([P, NBC], F32)
            e_part = pr.tile([P, KCH * NBC * ET], F32)
            e_mean2 = pr.tile([P, 2 * KCH], F32R)
            eeL_row = [pr.tile([1, HD], F32R, name=f"eeL{i}") for i in range(L)]

            # dram bounce buffers
            ag_in = [dram.tile([NPC, HID], F32R, name=f"ag_in{i}") for i in range(L)]
            h_table = [
                dram.tile([NTOT, HID], F32R, addr_space="Shared", name=f"h_table{i}")
                for i in range(L)
            ]
            er_in = dram.tile([KCH, P], F32)
            er_out = dram.tile([KCH, P], F32, addr_space="Shared")
            na_in = dram.tile([1, 1], F32)
            na_tot = dram.tile([1, 1], F32, addr_space="Shared")

            # ---------------- h0 = x @ W_in + (b_in + q@W_q + b_q)
            qrow_ps = ps_den.tile([1, HID], F32, tag="ps_den")
            for k in range(KCH):
                nc.tensor.matmul(out=qrow_ps[:], lhsT=qv[k][:], rhs=w_q[k][:],
                                 start=(k == 0), stop=(k == KCH - 1))
            qrow = cn.tile([1, HID], F32)
            bq_row = cn.tile([1, HID], F32)
            nc.sync.dma_start(out=bq_row[:], in_=bq_d[:, :])
            nc.vector.tensor_add(out=qrow[:], in0=bq_row[:], in1=qrow_ps[:])
            # broadcast [1,128] -> [128,128] via K=1 matmul
            bias_ps = ps_tr.tile([P, HID], F32, tag="ps_tr")
            qrow_r = cn.tile([1, HID], F32R)
            nc.vector.tensor_copy(out=qrow_r[:], in_=qrow[:])
            nc.tensor.matmul(out=bias_ps[:], lhsT=ones_row_r[:], rhs=qrow_r[:],
                             start=True, stop=True)
            bias_bc = cn.tile([P, HID], F32)
            nc.vector.tensor_copy(out=bias_bc[:], in_=bias_ps[:])

            for b in range(NBC):
                h0_ps = ps_xl.tile([P, HID], F32, tag="ps_xl")
                for k in range(KCH):
                    xk = wk.tile([P, P], F32R, tag="xk")
                    nc.sync.dma_start(out=xk[:], in_=xT_d[k * P:(k + 1) * P, b * P:(b + 1) * P])
                    nc.tensor.matmul(out=h0_ps[:], lhsT=xk[:], rhs=w_in[k][:],
                                     start=(k == 0), stop=(k == KCH - 1))
                nc.vector.tensor_add(out=h_node[:, b * P:(b + 1) * P], in0=bias_bc[:], in1=h0_ps[:])
                ht_ps = ps_tr.tile([P, P], F32, tag="ps_tr")
                nc.tensor.transpose(out=ht_ps[:], in_=h_node[:, b * P:(b + 1) * P], identity=ident_f[:])
                nc.vector.tensor_copy(out=hT[:, b * P:(b + 1) * P], in_=ht_ps[:])
                nc.gpsimd.dma_start(out=ag_in[0][b * P:(b + 1) * P, :],
                                    in_=h_node[:, b * P:(b + 1) * P])
            nc.gpsimd.collective_compute(
                "AllGather", OP.bypass, ins=[ag_in[0].opt()], outs=[h_table[0].opt()],
                replica_groups=groups)

            # ---------------- layers
            for i in range(L):
                # per-layer broadcast rows
                att_f = wd.tile([P, HD], F32, tag="attf")
                nc.sync.dma_start(out=att_f[:], in_=att_d[i, None, :].to_broadcast([P, HD]))
                att_bc = pr.tile([P, HD], BF16, name=f"att_bc{i}")
                nc.vector.tensor_copy(out=att_bc[:], in_=att_f[:])
                blr_bc = pr.tile([P, HD], F32, name=f"blr_bc{i}")
                nc.sync.dma_start(out=blr_bc[:], in_=blr_d[i, None, :].to_broadcast([P, HD]))
                blr_row_f = wd.tile([1, HD], F32, tag="blr_row_f")
                nc.sync.dma_start(out=blr_row_f[:], in_=blr_d[i, None, :])
                blr_row = pr.tile([1, HD], F32R, name=f"blr_row{i}")
                nc.vector.tensor_copy(out=blr_row[:], in_=blr_row_f[:])
                gb_bc = pr.tile([P, HID], F32, name=f"gb_bc{i}")
                nc.sync.dma_start(out=gb_bc[:], in_=gb_d[i, None, :].to_broadcast([P, HID]))
                lng_bc = pr.tile([P, HID], F32, name=f"lng_bc{i}")
                nc.sync.dma_start(out=lng_bc[:], in_=lng_d[i, None, :].to_broadcast([P, HID]))
                lnb_bc = pr.tile([P, HID], F32, name=f"lnb_bc{i}")
                nc.sync.dma_start(out=lnb_bc[:], in_=lnb_d[i, None, :].to_broadcast([P, HID]))

                # ---- edge phase
                for b in range(NBC):
                    xr_ps = ps_xl.tile([P, HD], F32, tag="ps_xl")
                    nc.tensor.matmul(out=xr_ps[:], lhsT=hT[:, b * P:(b + 1) * P],
                                     rhs=w_r[i][:], start=True, stop=True)
                    xr_sb = wd.tile([P, HD], F32R, tag="xr_sb", bufs=2)
                    nc.vector.tensor_add(out=xr_sb[:], in0=blr_bc[:], in1=xr_ps[:])

                    num_ps = ps_num.tile([P, HD], F32, tag="ps_num")
                    den_ps = ps_den.tile([P, 8], F32, tag="ps_den")

                    for t in range(ET):
                        gt_i = b * ET + t
                        eo = gt_i * P
                        first, last = (t == 0), (t == ET - 1)
                        srcc = wk.tile([P, 1], I32, tag="srcc")
                        nc.sync.dma_start(out=srcc[:], in_=src_d[eo:eo + P, None])
                        dstc = wk.tile([P, 1], I32, tag="dstc")
                        nc.sync.dma_start(out=dstc[:], in_=dstl_d[eo:eo + P, None])
                        dstrow = wk.tile([P, P], I32, tag="dstrow")
                        nc.sync.dma_start(out=dstrow[:],
                                          in_=dstl_d[eo:eo + P][None, :].to_broadcast([P, P]))
                        h_g = wk.tile([P, HID], F32R, tag="h_g")
                        nc.gpsimd.indirect_dma_start(
                            out=h_g[:], out_offset=None, in_=h_table[i][:],
                            in_offset=bass.IndirectOffsetOnAxis(ap=srcc[:, :1], axis=0))
                        ea_t = wk.tile([P, KCH * P], F32R, tag="ea_t")
                        nc.sync.dma_start(
                            out=ea_t[:].rearrange("p (k e) -> p k e", k=KCH),
                            in_=ea3_d[:].rearrange("p (k e) -> p k e", k=KCH)[:, :, eo:eo + P])

                        S_T = wk.tile([P, P], F32R, tag="S_T")
                        nc.vector.tensor_tensor(out=S_T[:], in0=dstc[:, :1].to_broadcast([P, P]),
                                                in1=iota_row[:], op=OP.is_equal)
                        S = wk.tile([P, P], F32R, tag="S")
                        nc.vector.tensor_tensor(out=S[:], in0=iota_col[:], in1=dstrow[:],
                                                op=OP.is_equal)
                        hgt_ps = ps_tr.tile([P, P], F32R, tag="ps_tr")
                        nc.tensor.transpose(out=hgt_ps[:], in_=h_g[:], identity=ident[:])
                        hT_g = wk.tile([P, HID], F32R, tag="hT_g")
                        nc.vector.tensor_copy(out=hT_g[:], in_=hgt_ps[:])

                        s_ps = ps_s.tile([P, HD], F32, tag="ps_s")
                        for k in range(KCH):
                            nc.tensor.matmul(out=s_ps[:], lhsT=ea_t[:, k * P:(k + 1) * P],
                                             rhs=w_e[i][k][:], start=(k == 0), stop=False)
                        nc.tensor.matmul(out=s_ps[:], lhsT=S[:], rhs=xr_sb[:],
                                         start=False, stop=False)
                        nc.tensor.matmul(out=s_ps[:], lhsT=hT_g[:], rhs=w_l[i][:],
                                         start=False, stop=True)
                        xl_ps = ps_xl.tile([P, HD], F32, tag="ps_xl")
                        nc.tensor.matmul(out=xl_ps[:], lhsT=hT_g[:], rhs=w_l[i][:],
                                         start=True, stop=True)

                        m = wd.tile([P, HD], BF16, tag="m")
                        nc.scalar.activation(out=m[:], in_=s_ps[:], func=AF.Prelu, alpha=NEG)
                        mt = wd.tile([P, HD], BF16, tag="mt")
                        nc.vector.tensor_mul(out=mt[:], in0=m[:], in1=att_bc[:])
                        logits = wk.tile([P, HEADS], F32, tag="logits")
                        nc.vector.reduce_sum(out=logits[:],
                                             in_=mt[:].rearrange("p (h c) -> p h c", h=HEADS),
                                             axis=AX.X)
                        exc = wk.tile([P, 6], F32R, tag="exc")
                        nc.scalar.activation(out=exc[:, :HEADS], in_=logits[:], func=AF.Exp)
                        nc.scalar.activation(out=exc[:, 4:6], in_=exc[:, 4:6], func=AF.Copy,
                                             scale=0.0, bias=1.0)
                        nc.tensor.matmul(out=den_ps[:, :6], lhsT=S_T[:], rhs=exc[:],
                                         start=first, stop=last)
                        exw = wd.tile([P, HD], F32R, tag="exw")
                        nc.vector.tensor_mul(
                            out=exw[:].rearrange("p (h c) -> p h c", h=HEADS),
                            in0=xl_ps[:].rearrange("p (h c) -> p h c", h=HEADS),
                            in1=exc[:, :HEADS][:, :, None].to_broadcast([P, HEADS, HID]))
                        nc.tensor.matmul(out=num_ps[:], lhsT=S_T[:], rhs=exw[:],
                                         start=first, stop=last)
                        if i == 0:
                            for k in range(KCH):
                                nc.vector.reduce_sum(
                                    out=e_part[:, k * NBC * ET + gt_i:k * NBC * ET + gt_i + 1],
                                    in_=ea_t[:, k * P:(k + 1) * P], axis=AX.X)

                    nc.vector.tensor_copy(out=num_all[:, b * HD:(b + 1) * HD], in_=num_ps[:])
                    nc.vector.tensor_copy(out=den_all[:, b * 5:(b + 1) * 5], in_=den_ps[:, :5])

                # ---- e_mean (layer 0 only) + ee_loop row
                if i == 0:
                    e_sum = pr.tile([P, KCH], F32)
                    for k in range(KCH):
                        nc.vector.reduce_sum(out=e_sum[:, k:k + 1],
                                             in_=e_part[:, k * NBC * ET:(k + 1) * NBC * ET],
                                             axis=AX.X)
                        nc.sync.dma_start(out=er_in[k, None, :], in_=e_sum[:, k:k + 1])
                    nc.gpsimd.collective_compute(
                        "AllReduce", OP.add, ins=[er_in.opt()], outs=[er_out.opt()],
                        replica_groups=groups)
                    e_full = pr.tile([P, KCH], F32)
                    e_full2 = pr.tile([P, 2 * KCH], F32)
                    for k in range(KCH):
                        nc.sync.dma_start(out=e_full[:, k:k + 1], in_=er_out[k, None, :])
                    nc.gpsimd.memset(e_full2[:], 0.0)
                    nc.vector.tensor_scalar_mul(
                        e_full2[:].rearrange("p (k two) -> p k two", two=2)[:, :, 0],
                        e_full[:], 1.0 / E)
                    nc.vector.tensor_copy(out=e_mean2[:], in_=e_full2[:])
                eeL_ps = ps_s.tile([2, HD], F32, tag="ps_s")
                for k in range(KCH):
                    nc.tensor.matmul(out=eeL_ps[:], lhsT=e_mean2[:, 2 * k:2 * k + 2],
                                     rhs=w_e[i][k][:],
                                     start=(k == 0), stop=(k == KCH - 1))
                nc.vector.tensor_copy(out=eeL_row[i][:], in_=eeL_ps[:1, :])

                # ---- node epilogue
                for b in range(NBC):
                    s_ps = ps_s.tile([P, HD], F32, tag="ps_s")
                    nc.tensor.matmul(out=s_ps[:], lhsT=hT[:, b * P:(b + 1) * P], rhs=w_l[i][:],
                                     start=True, stop=False)
                    nc.tensor.matmul(out=s_ps[:], lhsT=hT[:, b * P:(b + 1) * P], rhs=w_r[i][:],
                                     start=False, stop=False)
                    nc.tensor.matmul(out=s_ps[:], lhsT=ones_row_r[:], rhs=blr_row[:1, :],
                                     start=False, stop=False)
                    nc.tensor.matmul(out=s_ps[:], lhsT=ones_row_r[:], rhs=eeL_row[i][:],
                                     start=False, stop=True)
                    xl_ps = ps_xl.tile([P, HD], F32, tag="ps_xl")
                    nc.tensor.matmul(out=xl_ps[:], lhsT=hT[:, b * P:(b + 1) * P], rhs=w_l[i][:],
                                     start=True, stop=True)

                    m = wd.tile([P, HD], BF16, tag="m")
                    nc.scalar.activation(out=m[:], in_=s_ps[:], func=AF.Prelu, alpha=NEG)
                    mt = wd.tile([P, HD], BF16, tag="mt")
                    nc.vector.tensor_mul(out=mt[:], in0=m[:], in1=att_bc[:])
                    lgs = wk.tile([P, HEADS], F32, tag="logits")
                    nc.vector.reduce_sum(out=lgs[:],
                                         in_=mt[:].rearrange("p (h c) -> p h c", h=HEADS),
                                         axis=AX.X)
                    exs = wk.tile([P, HEADS], F32, tag="exs")
                    nc.scalar.activation(out=exs[:], in_=lgs[:], func=AF.Exp)

                    dent = wk.tile([P, HEADS], F32, tag="dent")
                    nc.vector.tensor_add(out=dent[:], in0=exs[:],
                                         in1=den_all[:, b * 5:b * 5 + 4])
                    rden = wk.tile([P, HEADS], F32, tag="rden")
                    nc.vector.reciprocal(rden[:], dent[:])
                    rden4 = wk.tile([P, HEADS], F32, tag="rden4")
                    nc.vector.tensor_scalar_mul(rden4[:], rden[:], 0.25)

                    t1 = wd.tile([P, HD], F32, tag="t1")
                    nc.vector.tensor_mul(
                        out=t1[:].rearrange("p (h c) -> p h c", h=HEADS),
                        in0=xl_ps[:].rearrange("p (h c) -> p h c", h=HEADS),
                        in1=exs[:][:, :, None].to_broadcast([P, HEADS, HID]))
                    t2 = wd.tile([P, HD], F32, tag="t2")
                    nc.vector.tensor_add(out=t2[:], in0=t1[:],
                                         in1=num_all[:, b * HD:(b + 1) * HD])
                    t3 = wd.tile([P, HD], F32, tag="t3")
                    nc.vector.tensor_mul(
                        out=t3[:].rearrange("p (h c) -> p h c", h=HEADS),
                        in0=t2[:].rearrange("p (h c) -> p h c", h=HEADS),
                        in1=rden4[:][:, :, None].to_broadcast([P, HEADS, HID]))
                    hmean = wk.tile([P, HID], F32, tag="hmean")
                    nc.vector.reduce_sum(out=hmean[:],
                                         in_=t3[:].rearrange("p (h c) -> p c h", h=HEADS),
                                         axis=AX.X)
                    hm2 = wk.tile([P, HID], F32, tag="hm2")
                    nc.vector.tensor_add(out=hm2[:], in0=hmean[:], in1=gb_bc[:])
                    # layernorm
                    mu = wk.tile([P, 1], F32, tag="mu")
                    nc.vector.reduce_sum(out=mu[:], in_=hm2[:], axis=AX.X)
                    nc.vector.tensor_scalar_mul(mu[:], mu[:], 1.0 / HID)
                    xc = wk.tile([P, HID], F32, tag="xc")
                    nc.vector.tensor_scalar(out=xc[:], in0=hm2[:], scalar1=mu[:, :1],
                                            scalar2=None, op0=OP.subtract)
                    sq = wk.tile([P, HID], F32, tag="sq")
                    nc.vector.tensor_mul(out=sq[:], in0=xc[:], in1=xc[:])
                    var = wk.tile([P, 1], F32, tag="var")
                    nc.vector.reduce_sum(out=var[:], in_=sq[:], axis=AX.X)
                    nc.vector.tensor_scalar(out=var[:], in0=var[:], scalar1=1.0 / HID,
                                            scalar2=EPS_LN, op0=OP.mult, op1=OP.add)
                    sd = wk.tile([P, 1], F32, tag="sd")
                    nc.scalar.activation(out=sd[:], in_=var[:], func=AF.Sqrt)
                    rstd = wk.tile([P, 1], F32, tag="rstd")
                    nc.vector.reciprocal(rstd[:], sd[:])
                    y = wk.tile([P, HID], F32, tag="y")
                    nc.vector.tensor_scalar(out=y[:], in0=xc[:], scalar1=rstd[:, :1],
                                            scalar2=None, op0=OP.mult)
                    y2 = wk.tile([P, HID], F32, tag="y2")
                    nc.vector.tensor_mul(out=y2[:], in0=y[:], in1=lng_bc[:])
                    y3 = wk.tile([P, HID], F32, tag="y3")
                    nc.vector.tensor_add(out=y3[:], in0=y2[:], in1=lnb_bc[:])
                    if i == 0:
                        hnew = wk.tile([P, HID], F32, tag="hnew")
                        nc.vector.tensor_add(out=hnew[:], in0=y3[:],
                                             in1=h_node[:, b * P:(b + 1) * P])
                        nc.scalar.activation(out=h_node[:, b * P:(b + 1) * P], in_=hnew[:],
                                             func=AF.Relu)
                        ht_ps = ps_tr.tile([P, P], F32, tag="ps_tr")
                        nc.tensor.transpose(out=ht_ps[:], in_=h_node[:, b * P:(b + 1) * P],
                                            identity=ident_f[:])
                        nc.vector.tensor_copy(out=hT[:, b * P:(b + 1) * P], in_=ht_ps[:])
                        nc.gpsimd.dma_start(out=ag_in[1][b * P:(b + 1) * P, :],
                                            in_=h_node[:, b * P:(b + 1) * P])
                    else:
                        nc.vector.tensor_add(out=h_node[:, b * P:(b + 1) * P], in0=y3[:],
                                             in1=h_node[:, b * P:(b + 1) * P])
                    # attention accumulation
                    at4 = wk.tile([P, HEADS], F32, tag="at4")
                    nc.vector.tensor_mul(out=at4[:], in0=den_all[:, b * 5:b * 5 + 4],
                                         in1=rden4[:])
                    al = wk.tile([P, 1], F32, tag="al")
                    nc.vector.reduce_sum(out=al[:], in_=at4[:], axis=AX.X)
                    if i == 0:
                        nc.vector.tensor_scalar_mul(attn_acc[:, b:b + 1], al[:], 0.5)
                    else:
                        al5 = wk.tile([P, 1], F32, tag="al5")
                        nc.vector.tensor_scalar_mul(al5[:], al[:], 0.5)
                        nc.vector.tensor_add(out=attn_acc[:, b:b + 1],
                                             in0=attn_acc[:, b:b + 1], in1=al5[:])

                if i == 0:
                    nc.gpsimd.collective_compute(
                        "AllGather", OP.bypass, ins=[ag_in[1].opt()], outs=[h_table[1].opt()],
                        replica_groups=groups)

            # ---------------- node_att finalize
            cnt = pr.tile([P, NBC], F32)
            nc.vector.tensor_copy(
                out=cnt[:],
                in_=den_all[:].rearrange("p (b f) -> p b f", f=5)[:, :, 4])
            nc.vector.tensor_scalar(out=cnt[:], in0=cnt[:], scalar1=1.0, scalar2=None,
                                    op0=OP.max)
            rc = pr.tile([P, NBC], F32)
            nc.vector.reciprocal(rc[:], cnt[:])
            na_raw = pr.tile([P, NBC], F32)
            nc.vector.tensor_mul(out=na_raw[:], in0=attn_acc[:], in1=rc[:])
            s1 = pr.tile([P, 1], F32)
            nc.vector.reduce_sum(out=s1[:], in_=na_raw[:], axis=AX.X)
            s1r = pr.tile([P, 2], F32R)
            nc.vector.tensor_copy(out=s1r[:], in_=s1[:, :1].to_broadcast([P, 2]))
            ones_col2 = pr.tile([P, 2], F32R)
            nc.vector.tensor_copy(out=ones_col2[:], in_=oc_f[:, :1].to_broadcast([P, 2]))
            tot_ps = ps_den.tile([2, 2], F32, tag="ps_den")
            nc.tensor.matmul(out=tot_ps[:], lhsT=s1r[:], rhs=ones_col2[:],
                             start=True, stop=True)
            tot_sb = pr.tile([1, 1], F32)
            nc.vector.tensor_copy(out=tot_sb[:], in_=tot_ps[:1, :1])
            nc.gpsimd.dma_start(out=na_in[:], in_=tot_sb[:])
            nc.gpsimd.collective_compute(
                "AllReduce", OP.add, ins=[na_in.opt()], outs=[na_tot.opt()],
                replica_groups=groups)
            totg = pr.tile([1, 1], F32)
            nc.gpsimd.dma_start(out=totg[:], in_=na_tot[:])
            nc.vector.tensor_scalar_add(totg[:], totg[:], 1e-8)
            rtot_f = pr.tile([1, 1], F32)
            nc.vector.reciprocal(rtot_f[:], totg[:])
            rtot = pr.tile([1, 2], F32R)
            nc.vector.tensor_copy(out=rtot[:], in_=rtot_f[:, :1].to_broadcast([1, 2]))
            rtot_ps = ps_den.tile([P, 2], F32, tag="ps_den")
            nc.tensor.matmul(out=rtot_ps[:], lhsT=ones_row_r[:], rhs=rtot[:],
                             start=True, stop=True)
            rtot_bc = pr.tile([P, 1], F32)
            nc.vector.tensor_copy(out=rtot_bc[:], in_=rtot_ps[:, :1])
            na_fin = pr.tile([P, NBC], F32)
            nc.vector.tensor_mul(out=na_fin[:], in0=na_raw[:],
                                 in1=rtot_bc[:, :1].to_broadcast([P, NBC]))

            # ---------------- outputs
            for b in range(NBC):
                nc.sync.dma_start(out=h_out_d[b * P:(b + 1) * P, :],
                                  in_=h_node[:, b * P:(b + 1) * P])
                nc.sync.dma_start(out=na_out_d[b * P:(b + 1) * P, None],
                                  in_=na_fin[:, b:b + 1])

    _split_waits(nc)
    return nc


# ------------------------------------------------------------ host wrapper
_prog_cache = {}


def prepare(x, edge_index, edge_attr, query_embedding,
            W_in, b_in, W_q, b_q, W_l, b_l, W_r, b_r, W_e, att,
            gat_bias, ln_g, ln_b):
    """Host prep: returns (nc, in_maps, postprocess) where postprocess maps
    per-core output dicts to the full (h, node_att)."""
    x = np.asarray(x, np.float32)
    edge_index = np.asarray(edge_index)
    edge_attr = np.asarray(edge_attr, np.float32)
    src = np.asarray(edge_index[0], np.int64)
    dst = np.asarray(edge_index[1], np.int64)

    # --- balance nodes into NB blocks of P slots (serpentine by degree)
    deg = np.bincount(dst, minlength=N)
    order = np.argsort(-deg, kind="stable")
    blk_of = np.empty(N, np.int32)
    slot_of = np.empty(N, np.int32)
    pos = 0
    rowcnt = np.zeros(NB, np.int32)
    for r in range(0, N, NB):
        chunk = order[r:r + NB]
        idxs = np.arange(len(chunk))
        bl = idxs if (r // NB) % 2 == 0 else NB - 1 - idxs
        blk_of[chunk] = bl
        slot_of[chunk] = rowcnt[bl]
        rowcnt[bl] += 1
    assert rowcnt.max() <= P
    new_id = blk_of.astype(np.int64) * P + slot_of  # orig -> new

    # --- per-block edge grouping
    eb = blk_of[dst]
    sort_idx = np.argsort(eb, kind="stable")
    eb_sorted = eb[sort_idx]
    blk_counts = np.bincount(eb_sorted, minlength=NB)
    ET = max(8, int(np.ceil(blk_counts.max() / P)))
    EPB = ET * P
    EPC = NBC * EPB
    blk_starts = np.concatenate([[0], np.cumsum(blk_counts)])[:-1]

    # slot arrays (padded)
    src_new = np.zeros(NB * EPB, np.int32)
    dstl = np.full(NB * EPB, 300, np.int32)
    ea_perm = np.zeros(NB * EPB, np.int64)   # orig edge index per slot
    ea_valid = np.zeros(NB * EPB, bool)
    for bl in range(NB):
        cnt = blk_counts[bl]
        sl = sort_idx[blk_starts[bl]:blk_starts[bl] + cnt]
        base = bl * EPB
        src_new[base:base + cnt] = new_id[src[sl]]
        dstl[base:base + cnt] = slot_of[dst[sl]]
        ea_perm[base:base + cnt] = sl
        ea_valid[base:base + cnt] = True

    # --- per-core inputs
    Wl = np.asarray(W_l, np.float32)
    Wr = np.asarray(W_r, np.float32)
    We = np.asarray(W_e, np.float32)
    attv = np.asarray(att, np.float32).reshape(L, HD)
    blr = (np.asarray(b_l, np.float32) + np.asarray(b_r, np.float32)).reshape(L, HD)
    binq = (np.asarray(b_in, np.float32) + np.asarray(b_q, np.float32)).reshape(1, HID)
    shared = {
        "w_in": np.ascontiguousarray(np.asarray(W_in, np.float32)),
        "w_q": np.ascontiguousarray(np.asarray(W_q, np.float32)),
        "qv": np.ascontiguousarray(np.asarray(query_embedding, np.float32).reshape(DIN, 1)),
        "b_inq": binq,
        "w_l": Wl, "w_r": Wr, "w_e": We,
        "attv": attv, "blr": blr,
        "gb": np.asarray(gat_bias, np.float32),
        "lng": np.asarray(ln_g, np.float32),
        "lnb": np.asarray(ln_b, np.float32),
    }
    in_maps = []
    inv_new = np.full(NTOT, -1, np.int64)
    inv_new[new_id] = np.arange(N)
    for c in range(NC):
        sl = slice(c * EPC, (c + 1) * EPC)
        perm_c = ea_perm[sl]
        valid_c = ea_valid[sl]
        ea_c = edge_attr[perm_c]            # [EPC, DIN]
        ea_c[~valid_c] = 0.0
        # ea3 layout: [128, KCH*EPC]; ea3[p, k*EPC + e] = ea_c[e, k*128+p]
        ea3 = np.ascontiguousarray(
            ea_c.reshape(EPC, KCH, P).transpose(2, 1, 0).reshape(P, KCH * EPC))
        nodes_c = inv_new[c * NPC:(c + 1) * NPC]
        xT = np.zeros((DIN, NPC), np.float32)
        m = nodes_c >= 0
        xT[:, m] = x[nodes_c[m]].T
        in_maps.append({
            "ea3": ea3,
            "srcs": np.ascontiguousarray(src_new[sl]),
            "dstl": np.ascontiguousarray(dstl[sl]),
            "xT": xT,
            **shared,
        })

    key = ET
    if key not in _prog_cache:
        _prog_cache[key] = build_program(ET)
    nc = _prog_cache[key]

    def postprocess(outs):
        h_full = np.zeros((N, HID), np.float32)
        na_full = np.zeros((N,), np.float32)
        for c in range(NC):
            nodes_c = inv_new[c * NPC:(c + 1) * NPC]
            m = nodes_c >= 0
            h_full[nodes_c[m]] = outs[c]["h_out"][m]
            na_full[nodes_c[m]] = outs[c]["na_out"][m]
        return h_full, na_full

    return nc, in_maps, postprocess


def kernel(**inputs):
    from concourse.bass_utils import run_bass_kernel_spmd

    nc, in_maps, post = prepare(**inputs)
    res = run_bass_kernel_spmd(nc, in_maps, list(range(NC)))
    return post(res.results)
